# revision 1
# baseline (speedup 1.0000x reference)
"""Two-layer GCN (PyG GCNConv x2 with relu between) on 8 Trainium2 NeuronCores.

Math (per layer, A' = D^-1/2 (A + I) D^-1/2):
    h  = relu(A' (z @ W1) + b1)
    out = A' (h @ W2) + b2  ==  (A' h) @ W2 + b2      (aggregation commutes with the
                                                       feature-space linear map)
Both layers therefore aggregate 128-wide features only.

Distribution: nodes (and dst-partitioned edges) sharded across 8 cores;
weights replicated; per-layer AllGather of the (dinv-scaled) feature table in
bf16; per-core gather of source rows via bulk SWDGE dma_gather; segment-sum
realized as one-hot matmuls accumulating in PSUM.

The Bass program is specialized to the actual graph: per-(window, class)
chunk counts are compile-time constants derived from edge_index.
"""

import numpy as np
import ml_dtypes

P = 128
NCORES = 8
NCLASS = 4          # src-range classes so relative gather indices fit int16
G = 4               # dst windows per gather group

BF16 = ml_dtypes.bfloat16

_PROGRAM_CACHE = {}


# ----------------------------------------------------------------- host prep


def _plan(edge_index, N):
    """Sort/partition edges; all compile-time metadata + per-core slot arrays."""
    WPC = -(-N // (NCORES * P))            # windows per core
    SHARD = WPC * P
    NPAD = NCORES * SHARD
    CLS = NPAD // NCLASS
    assert NPAD % NCLASS == 0 and CLS <= 32768

    src = np.asarray(edge_index[0], dtype=np.int64)
    dst = np.asarray(edge_index[1], dtype=np.int64)
    deg = np.bincount(dst, minlength=N).astype(np.float64) + 1.0

    loops = np.arange(N, dtype=np.int64)
    s2 = np.concatenate([src, loops])
    d2 = np.concatenate([dst, loops])

    win = d2 >> 7
    cls = s2 // CLS
    key = win * NCLASS + cls
    order = np.argsort(key, kind="stable")
    s2s = s2[order]
    d2s = d2[order]

    NW = NPAD // P
    cellcnt = np.bincount(key, minlength=NW * NCLASS)
    cellstart = np.concatenate([[0], np.cumsum(cellcnt)]).astype(np.int64)
    counts_core = cellcnt.reshape(NCORES, WPC, NCLASS)
    chunks_wc = -(-counts_core.max(axis=0) // P)      # [WPC, NCLASS]

    groups = [list(range(g, min(g + G, WPC))) for g in range(0, WPC, G)]

    # global chunk layout: for each group, for each class, for each window in
    # group, that window's class chunks (one contiguous dma_gather per
    # (group, class)).
    group_meta = []           # per group: dict with chunk/col offsets
    wmeta = [dict(schunks=[], gchunks=[]) for _ in range(WPC)]
    chunkpos = 0
    colpos = 0
    cell_slot = {}            # (w, c) -> global slot start
    for grp in groups:
        g_chunk_base = chunkpos
        g_col_base = colpos
        calls = []
        for c in range(NCLASS):
            call_chunk_start = chunkpos
            call_col_start = colpos
            for w in grp:
                ncw = int(chunks_wc[w, c])
                cell_slot[(w, c)] = chunkpos * P
                chunkpos += ncw
            cn = chunkpos - call_chunk_start
            colpos += cn * P // 16
            calls.append(dict(chunk_start=call_chunk_start, nchunks=cn,
                              col_start=call_col_start, ncols=colpos - call_col_start))
        group_meta.append(dict(chunk_base=g_chunk_base, nchunks=chunkpos - g_chunk_base,
                               col_base=g_col_base, ncols=colpos - g_col_base,
                               calls=calls, windows=list(grp)))
    TOTCHUNKS = chunkpos
    TOTCOLS = colpos

    # per-window ordered chunk lists: s-order (class-major) + matching global
    # chunk ids, and per-(w,c) count for S generation
    for w in range(WPC):
        for c in range(NCLASS):
            ncw = int(chunks_wc[w, c])
            if ncw == 0:
                continue
            base = cell_slot[(w, c)] // P
            wmeta[w]["gchunks"].extend(range(base, base + ncw))
            wmeta[w]["schunks"].append((c, ncw, base))

    # per-core slot arrays
    idx_slots = np.zeros((NCORES, TOTCHUNKS * P), np.int16)
    dst_slots = np.full((NCORES, TOTCHUNKS * P), 300.0, np.float32)
    for w in range(WPC):
        for c in range(NCLASS):
            ncw = int(chunks_wc[w, c])
            if ncw == 0:
                continue
            s0 = cell_slot[(w, c)]
            for core in range(NCORES):
                cidx = (core * WPC + w) * NCLASS + c
                cnt = int(cellcnt[cidx])
                st = int(cellstart[cidx])
                idx_slots[core, s0:s0 + cnt] = (s2s[st:st + cnt] - c * CLS).astype(np.int16)
                dst_slots[core, s0:s0 + cnt] = (d2s[st:st + cnt] & (P - 1)).astype(np.float32)

    # wrapped int16 index tensors (per call: idx i at [i%16, i//16], tiled x8)
    idx16 = np.zeros((NCORES, 128, TOTCOLS), np.int16)
    for gm in group_meta:
        for call in gm["calls"]:
            cn = call["nchunks"]
            if cn == 0:
                continue
            s0 = call["chunk_start"] * P
            c0 = call["col_start"]
            seg = idx_slots[:, s0:s0 + cn * P]                  # [NCORES, n]
            wrapped = seg.reshape(NCORES, cn * P // 16, 16).transpose(0, 2, 1)
            idx16[:, :, c0:c0 + cn * P // 16] = np.tile(wrapped, (1, 8, 1))

    dstloc = dst_slots.reshape(NCORES, TOTCHUNKS, P).transpose(0, 2, 1)  # [NCORES,128,TOTCHUNKS]

    meta = dict(N=N, WPC=WPC, SHARD=SHARD, NPAD=NPAD, CLS=CLS,
                TOTCHUNKS=TOTCHUNKS, TOTCOLS=TOTCOLS,
                groups=group_meta, wmeta=wmeta,
                chunks_sig=chunks_wc.tobytes())
    return meta, deg, idx16, dstloc.astype(BF16)


# ------------------------------------------------------------- bass program


def _build_program(meta, IN_C, HID, OUT_C, debug_phase=None):
    import concourse.bacc as bacc
    import concourse.mybir as mybir
    import concourse.tile as tile

    WPC, SHARD, NPAD, CLS = meta["WPC"], meta["SHARD"], meta["NPAD"], meta["CLS"]
    TOTCHUNKS, TOTCOLS = meta["TOTCHUNKS"], meta["TOTCOLS"]
    KIN = IN_C // P

    nc = bacc.Bacc("TRN2", target_bir_lowering=False, debug=False,
                   num_devices=NCORES)
    f32, bf16, i16, i32 = (mybir.dt.float32, mybir.dt.bfloat16,
                           mybir.dt.int16, mybir.dt.int32)

    z_shard = nc.dram_tensor("z_shard", [SHARD, IN_C], bf16, kind="ExternalInput").ap()
    w1 = nc.dram_tensor("w1", [IN_C, HID], bf16, kind="ExternalInput").ap()
    w2 = nc.dram_tensor("w2", [HID, OUT_C], bf16, kind="ExternalInput").ap()
    idx16 = nc.dram_tensor("idx16", [128, TOTCOLS], i16, kind="ExternalInput").ap()
    dstloc = nc.dram_tensor("dstloc", [128, TOTCHUNKS], bf16, kind="ExternalInput").ap()
    dinv_col = nc.dram_tensor("dinv_col", [P, WPC], f32, kind="ExternalInput").ap()
    sqd_row = nc.dram_tensor("sqd_row", [1, SHARD], bf16, kind="ExternalInput").ap()
    b1r = nc.dram_tensor("b1r", [1, HID], bf16, kind="ExternalInput").ap()
    b2r = nc.dram_tensor("b2r", [1, OUT_C], bf16, kind="ExternalInput").ap()
    out_shard = nc.dram_tensor("out_shard", [SHARD, OUT_C], f32, kind="ExternalOutput").ap()
    dbg = None
    if debug_phase is not None:
        dbg = nc.dram_tensor("dbg", [NPAD, HID], bf16, kind="ExternalOutput").ap()

    with tile.TileContext(nc) as tc:
        with (
            tc.tile_pool(name="dram", bufs=1, space="DRAM") as dram,
            tc.tile_pool(name="const", bufs=1) as cp,
        ):
            ag1_in = dram.tile([SHARD, HID], bf16)
            table1 = dram.tile([NPAD, HID], bf16, addr_space="Shared")
            ag2_in = dram.tile([SHARD, HID], bf16)
            table2 = dram.tile([NPAD, HID], bf16, addr_space="Shared")

            w1sb = cp.tile([P, KIN * HID], bf16)
            for ic in range(KIN):
                nc.sync.dma_start(w1sb[:, ic * HID:(ic + 1) * HID],
                                  w1[ic * P:(ic + 1) * P, :])
            w2sb = cp.tile([P, OUT_C], bf16)
            nc.sync.dma_start(w2sb[:], w2[:])
            dinvsb = cp.tile([P, WPC], f32)
            nc.sync.dma_start(dinvsb[:], dinv_col[:])
            sqdsb = cp.tile([1, SHARD], bf16)
            nc.sync.dma_start(sqdsb[:], sqd_row[:])
            b1sb = cp.tile([1, HID], bf16)
            nc.sync.dma_start(b1sb[:], b1r[:])
            b2sb = cp.tile([1, OUT_C], bf16)
            nc.sync.dma_start(b2sb[:], b2r[:])

            iota_i = cp.tile([P, P], i32)
            nc.gpsimd.iota(iota_i[:], pattern=[[1, P]], base=0, channel_multiplier=0)
            iota_bf = cp.tile([P, P], bf16)
            nc.vector.tensor_copy(iota_bf[:], iota_i[:])

            # ---------------- phase A: h1' = (z @ W1) * dinv  (own shard)
            with (
                tc.tile_pool(name="mmA", bufs=2) as mp,
                tc.tile_pool(name="psA", bufs=2, space="PSUM") as psA,
            ):
                for t0 in range(0, SHARD, 512):
                    gsz = min(512, SHARD - t0)
                    zts = []
                    for ic in range(KIN):
                        zt = mp.tile([P, gsz], bf16, tag=f"zt{ic}",
                                     padded_shape=[P, 512], name=f"zt{ic}")
                        nc.sync.dma_start_transpose(
                            zt[:], z_shard[t0:t0 + gsz, ic * P:(ic + 1) * P])
                        zts.append(zt)
                    for sub in range(gsz // P):
                        nt = t0 // P + sub
                        ps = psA.tile([P, HID], f32, name="psa")
                        for ic in range(KIN):
                            nc.tensor.matmul(
                                ps[:], lhsT=zts[ic][:, sub * P:(sub + 1) * P],
                                rhs=w1sb[:, ic * HID:(ic + 1) * HID],
                                start=(ic == 0), stop=(ic == KIN - 1))
                        hsb = mp.tile([P, HID], bf16, tag="hsb", name="hsb")
                        nc.scalar.mul(hsb[:], ps[:], dinvsb[:, nt:nt + 1])
                        nc.sync.dma_start(ag1_in[nt * P:(nt + 1) * P, :], hsb[:])

            nc.gpsimd.collective_compute(
                "AllGather", mybir.AluOpType.bypass,
                replica_groups=[list(range(NCORES))],
                ins=[ag1_in[:]], outs=[table1[:]])

            # ---------------- aggregation layers
            def agg_layer(table, layer, dbg_mode=None):
                with (
                    tc.tile_pool(name=f"gat{layer}", bufs=2) as gp,
                    tc.tile_pool(name=f"s{layer}", bufs=3) as sp,
                    tc.tile_pool(name=f"eps{layer}", bufs=3) as ep,
                    tc.tile_pool(name=f"ps{layer}", bufs=2, space="PSUM") as pp,
                    tc.tile_pool(name=f"pso{layer}", bufs=2, space="PSUM") as po,
                ):
                    maxgch = max(gm["nchunks"] for gm in meta["groups"])
                    maxgcol = max(gm["ncols"] for gm in meta["groups"])
                    maxsch = max(len(wm["gchunks"]) for wm in meta["wmeta"])
                    for gm in meta["groups"]:
                        gch, gcol = gm["nchunks"], gm["ncols"]
                        idx_sb = gp.tile([128, gcol], i16, tag="idx",
                                         padded_shape=[128, maxgcol], name="idx_sb")
                        nc.sync.dma_start(idx_sb[:], idx16[:, gm["col_base"]:gm["col_base"] + gcol])
                        dl_sb = gp.tile([P, gch], bf16, tag="dl",
                                        padded_shape=[P, maxgch], name="dl_sb")
                        nc.sync.dma_start(dl_sb[:], dstloc[:, gm["chunk_base"]:gm["chunk_base"] + gch])
                        gbuf = gp.tile([P, gch * P], bf16, tag="gbuf",
                                       padded_shape=[P, maxgch * P], name="gbuf")
                        for c, call in enumerate(gm["calls"]):
                            cn = call["nchunks"]
                            if cn == 0:
                                continue
                            off = call["chunk_start"] - gm["chunk_base"]
                            loc0 = call["col_start"] - gm["col_base"]
                            # <=1024 idx per call: the single_packet fast path
                            # (64-desc packet x 16 lanes) without ring stalls
                            for s0 in range(0, cn, 8):
                                sc = min(8, cn - s0)
                                nc.gpsimd.dma_gather(
                                    out_ap=gbuf[:, (off + s0) * P:(off + s0 + sc) * P]
                                        .rearrange("p (k f) -> p k f", f=P),
                                    in_ap=table[c * CLS:, :],
                                    idxs_ap=idx_sb[:, loc0 + s0 * 8:loc0 + (s0 + sc) * 8],
                                    num_idxs=sc * P,
                                    num_idxs_reg=sc * P,
                                    elem_size=HID,
                                    single_packet=True,
                                )
                        if dbg_mode == "gather":
                            # consume gbuf: copy first window-tile to ag2_in
                            gcp = ep.tile([P, HID], bf16, tag="l1", name="gcp")
                            nc.vector.tensor_copy(gcp[:], gbuf[:, :HID])
                            nc.sync.dma_start(
                                ag2_in[gm["windows"][0] * P:(gm["windows"][0] + 1) * P, :],
                                gcp[:])
                            continue
                        for w in gm["windows"]:
                            wm = meta["wmeta"][w]
                            cw = len(wm["gchunks"])
                            s_sb = sp.tile([P, max(cw, 1) * P], bf16, tag="s",
                                           padded_shape=[P, maxsch * P], name="s_sb")
                            soff = 0
                            for (c, ncw, gbase) in wm["schunks"]:
                                lc0 = gbase - gm["chunk_base"]
                                in0 = (dl_sb[:, lc0:lc0 + ncw]
                                       .rearrange("p (c one) -> p c one", one=1)
                                       .to_broadcast([P, ncw, P]))
                                in1 = (iota_bf[:]
                                       .rearrange("p (one j) -> p one j", one=1)
                                       .to_broadcast([P, ncw, P]))
                                nc.vector.tensor_tensor(
                                    out=s_sb[:, soff * P:(soff + ncw) * P]
                                        .rearrange("p (c j) -> p c j", j=P),
                                    in0=in0, in1=in1,
                                    op=mybir.AluOpType.is_equal)
                                soff += ncw
                            ps = pp.tile([P, P], f32, name="ps")
                            if layer == 1:
                                use_bias = dbg_mode != "nobias"
                                if use_bias:
                                    nc.tensor.matmul(
                                        ps[:], lhsT=sqdsb[:, w * P:(w + 1) * P],
                                        rhs=b1sb[:], start=True, stop=(cw == 0))
                                for j, gc in enumerate(wm["gchunks"]):
                                    lgc = gc - gm["chunk_base"]
                                    nc.tensor.matmul(
                                        ps[:],
                                        lhsT=s_sb[:, j * P:(j + 1) * P],
                                        rhs=gbuf[:, lgc * P:(lgc + 1) * P],
                                        start=(not use_bias and j == 0),
                                        stop=(j == cw - 1))
                                l1sb = ep.tile([P, HID], bf16, tag="l1", name="l1sb")
                                if dbg_mode == "nobias":
                                    nc.vector.tensor_copy(l1sb[:], ps[:])
                                    nc.sync.dma_start(ag2_in[w * P:(w + 1) * P, :], l1sb[:])
                                    continue
                                nc.scalar.activation(
                                    l1sb[:], ps[:],
                                    mybir.ActivationFunctionType.Relu,
                                    scale=dinvsb[:, w:w + 1])
                                l2row = ep.tile([P, HID], bf16, tag="l2r", name="l2row")
                                nc.vector.tensor_scalar(
                                    out=l2row[:], in0=l1sb[:],
                                    scalar1=dinvsb[:, w:w + 1], scalar2=None,
                                    op0=mybir.AluOpType.mult)
                                nc.sync.dma_start(ag2_in[w * P:(w + 1) * P, :], l2row[:])
                            else:
                                # transposed accumulate: ps[f, d]
                                for j, gc in enumerate(wm["gchunks"]):
                                    lgc = gc - gm["chunk_base"]
                                    nc.tensor.matmul(
                                        ps[:],
                                        lhsT=gbuf[:, lgc * P:(lgc + 1) * P],
                                        rhs=s_sb[:, j * P:(j + 1) * P],
                                        start=(j == 0), stop=(j == cw - 1))
                                a2t = ep.tile([P, P], bf16, tag="a2t", name="a2t")
                                if cw == 0:
                                    nc.vector.memset(a2t[:], 0)
                                else:
                                    nc.vector.tensor_copy(a2t[:], ps[:])
                                ops = po.tile([P, OUT_C], f32, name="ops")
                                nc.tensor.matmul(ops[:], lhsT=a2t[:], rhs=w2sb[:],
                                                 start=True, stop=False)
                                nc.tensor.matmul(ops[:], lhsT=sqdsb[:, w * P:(w + 1) * P],
                                                 rhs=b2sb[:], start=False, stop=True)
                                fsb = ep.tile([P, OUT_C], f32, tag="fout", name="fsb")
                                nc.scalar.mul(fsb[:], ops[:], dinvsb[:, w:w + 1])
                                nc.sync.dma_start(out_shard[w * P:(w + 1) * P, :], fsb[:])

            if debug_phase == "A":
                nc.sync.dma_start(dbg[:], table1[:])
            else:
                agg_layer(table1[:], 1,
                          dbg_mode=debug_phase if debug_phase in ("gather", "nobias") else None)
                if debug_phase in ("C1", "gather", "nobias"):
                    nc.sync.dma_start(dbg[:SHARD, :], ag2_in[:])
                else:
                    nc.gpsimd.collective_compute(
                        "AllGather", mybir.AluOpType.bypass,
                        replica_groups=[list(range(NCORES))],
                        ins=[ag2_in[:]], outs=[table2[:]])
                    agg_layer(table2[:], 2)

    nc.compile()
    return nc


# ----------------------------------------------------------------- entry


def _prepare_and_build(z, edge_index, W1, b1, W2, b2):
    N, IN_C = z.shape
    HID = W1.shape[1]
    OUT_C = W2.shape[1]
    meta, deg, idx16, dstloc = _plan(edge_index, N)
    WPC, SHARD, NPAD = meta["WPC"], meta["SHARD"], meta["NPAD"]

    dinv = (1.0 / np.sqrt(deg)).astype(np.float32)
    dinv_pad = np.zeros(NPAD, np.float32)
    dinv_pad[:N] = dinv
    sqd_pad = np.zeros(NPAD, np.float32)
    sqd_pad[:N] = np.sqrt(deg).astype(np.float32)

    zpad = np.zeros((NPAD, IN_C), BF16)
    zpad[:N] = z.astype(BF16)

    w1b = np.ascontiguousarray(W1.astype(BF16))
    w2b = np.ascontiguousarray(W2.astype(BF16))
    b1b = np.ascontiguousarray(b1.reshape(1, HID).astype(BF16))
    b2b = np.ascontiguousarray(b2.reshape(1, OUT_C).astype(BF16))

    in_maps = []
    for c in range(NCORES):
        sl = slice(c * SHARD, (c + 1) * SHARD)
        in_maps.append({
            "z_shard": np.ascontiguousarray(zpad[sl]),
            "w1": w1b, "w2": w2b,
            "idx16": np.ascontiguousarray(idx16[c]),
            "dstloc": np.ascontiguousarray(dstloc[c]),
            "dinv_col": np.ascontiguousarray(dinv_pad[sl].reshape(WPC, P).T),
            "sqd_row": np.ascontiguousarray(sqd_pad[sl].reshape(1, SHARD).astype(BF16)),
            "b1r": b1b, "b2r": b2b,
        })

    cache_key = (N, IN_C, HID, OUT_C, meta["TOTCHUNKS"], hash(meta["chunks_sig"]))
    if cache_key in _PROGRAM_CACHE:
        nc = _PROGRAM_CACHE[cache_key]
    else:
        nc = _build_program(meta, IN_C, HID, OUT_C)
        _PROGRAM_CACHE[cache_key] = nc
    return nc, in_maps, meta


def _run(inputs, trace=False, trace_kwargs=None):
    from concourse.bass_utils import run_bass_kernel_spmd

    z = np.asarray(inputs["z"])
    edge_index = np.asarray(inputs["edge_index"])
    W1 = np.asarray(inputs["W1"])
    b1 = np.asarray(inputs["b1"])
    W2 = np.asarray(inputs["W2"])
    b2 = np.asarray(inputs["b2"])

    nc, in_maps, meta = _prepare_and_build(z, edge_index, W1, b1, W2, b2)
    res = run_bass_kernel_spmd(
        nc, in_maps, core_ids=list(range(NCORES)),
        trace=trace, **(trace_kwargs or {}))
    N = meta["N"]
    out = np.concatenate([r["out_shard"] for r in res.results], axis=0)[:N]
    return np.ascontiguousarray(out.astype(np.float32)), res


def kernel(**inputs):
    out, _ = _run(inputs, trace=False)
    return out



# revision 11
# speedup vs baseline: 2.2797x; 2.2797x over previous
"""Two-layer GCN (PyG GCNConv x2 with relu between) on 8 Trainium2 NeuronCores.

Math (per layer, A' = D^-1/2 (A + I) D^-1/2):
    h  = relu(A' (z @ W1) + b1)
    out = A' (h @ W2) + b2  ==  (A' h) @ W2 + b2      (aggregation commutes with the
                                                       feature-space linear map)
Both layers therefore aggregate 128-wide features only.

Distribution: nodes (and dst-partitioned edges) sharded across 8 cores;
weights replicated; per-layer AllGather of the (dinv-scaled) feature table in
bf16; per-core gather of source rows via bulk SWDGE dma_gather; segment-sum
realized as one-hot matmuls accumulating in PSUM.

Perf structure (vs the naive version):
  * dma_gather descriptor generation runs on ONE Q7 core pair selected by
    queue_num; with num_swdge_queues=4 and gathers spread round-robin over
    queues 0-3, four core pairs generate descriptors concurrently
    (~2.6 ns/idx instead of ~8.8).
  * The per-layer AllGather is split into 4 class chunks (classes = row
    quarters of each shard, permuted table layout) so gathers for class c
    only wait on AG chunk c, and AG chunks issue as soon as the producing
    quarter of the shard is computed -> collectives overlap compute.

The Bass program is specialized to the actual graph: per-(window, class)
chunk counts are compile-time constants derived from edge_index.
"""

import numpy as np
import ml_dtypes

P = 128
NCORES = 8
NCLASS = 4          # src-range classes (= AllGather chunks); rel idx fits int16
G = 4               # dst windows per gather group

BF16 = ml_dtypes.bfloat16

_PROGRAM_CACHE = {}


# ----------------------------------------------------------------- host prep


def _plan(edge_index, N):
    """Sort/partition edges; all compile-time metadata + per-core slot arrays."""
    WPC = -(-N // (NCORES * P))            # windows per core
    SHARD = WPC * P
    NPAD = NCORES * SHARD

    # quarters of each shard in WINDOWS (so AG chunk boundaries align with
    # aggregation window groups); table layout is class-major:
    #   node (core c, local row r) with r in quarter j sits at table row
    #   CLS_BASE[j] + c * QROWS[j] + (r - QSTART[j])
    wq = WPC // NCLASS
    QWIN = [wq + (1 if j < WPC % NCLASS else 0) for j in range(NCLASS)]
    QROWS = [w * P for w in QWIN]
    QSTART = np.concatenate([[0], np.cumsum(QROWS)]).astype(np.int64)
    CLS = [NCORES * q for q in QROWS]
    CLS_BASE = np.concatenate([[0], np.cumsum(CLS)]).astype(np.int64)
    assert max(CLS) <= 32768

    src = np.asarray(edge_index[0], dtype=np.int64)
    dst = np.asarray(edge_index[1], dtype=np.int64)
    deg = np.bincount(dst, minlength=N).astype(np.float64) + 1.0

    loops = np.arange(N, dtype=np.int64)
    s2 = np.concatenate([src, loops])
    d2 = np.concatenate([dst, loops])

    # class + within-class (permuted-table) index of each source node
    s_core = s2 // SHARD
    s_r = s2 % SHARD
    s_cls = np.searchsorted(QSTART, s_r, side="right") - 1      # quarter of r
    s_rel = s_core * np.array(QROWS)[s_cls] + (s_r - QSTART[s_cls])

    win = d2 >> 7
    key = win * NCLASS + s_cls
    order = np.argsort(key, kind="stable")
    rel_s = s_rel[order]
    d2s = d2[order]

    NW = NPAD // P
    cellcnt = np.bincount(key, minlength=NW * NCLASS)
    cellstart = np.concatenate([[0], np.cumsum(cellcnt)]).astype(np.int64)
    counts_core = cellcnt.reshape(NCORES, WPC, NCLASS)
    chunks_wc = -(-counts_core.max(axis=0) // P)      # [WPC, NCLASS]

    groups = [list(range(g, min(g + G, WPC))) for g in range(0, WPC, G)]

    # global chunk layout: for each group, for each class, for each window in
    # group, that window's class chunks (sub-calls of <=8 chunks per
    # (group, class); queue_num = class).
    group_meta = []           # per group: dict with chunk/col offsets
    wmeta = [dict(schunks=[], gchunks=[]) for _ in range(WPC)]
    chunkpos = 0
    colpos = 0
    cell_slot = {}            # (w, c) -> global slot start
    for grp in groups:
        g_chunk_base = chunkpos
        g_col_base = colpos
        calls = []
        for c in range(NCLASS):
            call_chunk_start = chunkpos
            call_col_start = colpos
            for w in grp:
                ncw = int(chunks_wc[w, c])
                cell_slot[(w, c)] = chunkpos * P
                chunkpos += ncw
            cn = chunkpos - call_chunk_start
            colpos += cn * P // 16
            calls.append(dict(chunk_start=call_chunk_start, nchunks=cn,
                              col_start=call_col_start, ncols=colpos - call_col_start,
                              cls=c))
        group_meta.append(dict(chunk_base=g_chunk_base, nchunks=chunkpos - g_chunk_base,
                               col_base=g_col_base, ncols=colpos - g_col_base,
                               calls=calls, windows=list(grp)))
    TOTCHUNKS = chunkpos
    TOTCOLS = colpos

    # per-window ordered chunk lists: s-order (class-major) + matching global
    # chunk ids, and per-(w,c) count for S generation
    for w in range(WPC):
        for c in range(NCLASS):
            ncw = int(chunks_wc[w, c])
            if ncw == 0:
                continue
            base = cell_slot[(w, c)] // P
            wmeta[w]["gchunks"].extend(range(base, base + ncw))
            wmeta[w]["schunks"].append((c, ncw, base))

    # per-core slot arrays
    idx_slots = np.zeros((NCORES, TOTCHUNKS * P), np.int16)
    dst_slots = np.full((NCORES, TOTCHUNKS * P), 300.0, np.float32)
    for w in range(WPC):
        for c in range(NCLASS):
            ncw = int(chunks_wc[w, c])
            if ncw == 0:
                continue
            s0 = cell_slot[(w, c)]
            for core in range(NCORES):
                cidx = (core * WPC + w) * NCLASS + c
                cnt = int(cellcnt[cidx])
                st = int(cellstart[cidx])
                idx_slots[core, s0:s0 + cnt] = rel_s[st:st + cnt].astype(np.int16)
                dst_slots[core, s0:s0 + cnt] = (d2s[st:st + cnt] & (P - 1)).astype(np.float32)

    # wrapped int16 index tensors (per call: idx i at [i%16, i//16], tiled x8)
    idx16 = np.zeros((NCORES, 128, TOTCOLS), np.int16)
    for gm in group_meta:
        for call in gm["calls"]:
            cn = call["nchunks"]
            if cn == 0:
                continue
            s0 = call["chunk_start"] * P
            c0 = call["col_start"]
            seg = idx_slots[:, s0:s0 + cn * P]                  # [NCORES, n]
            wrapped = seg.reshape(NCORES, cn * P // 16, 16).transpose(0, 2, 1)
            idx16[:, :, c0:c0 + cn * P // 16] = np.tile(wrapped, (1, 8, 1))

    dstloc = dst_slots.reshape(NCORES, TOTCHUNKS, P).transpose(0, 2, 1)  # [NCORES,128,TOTCHUNKS]

    meta = dict(N=N, WPC=WPC, SHARD=SHARD, NPAD=NPAD,
                QWIN=QWIN, QROWS=QROWS, QSTART=[int(x) for x in QSTART],
                CLS=CLS, CLS_BASE=[int(x) for x in CLS_BASE],
                TOTCHUNKS=TOTCHUNKS, TOTCOLS=TOTCOLS,
                groups=group_meta, wmeta=wmeta,
                chunks_sig=chunks_wc.tobytes())
    return meta, deg, idx16, dstloc.astype(BF16)


# ------------------------------------------------------------- bass program


def _build_program(meta, IN_C, HID, OUT_C):
    import concourse.bacc as bacc
    import concourse.mybir as mybir
    import concourse.tile as tile

    WPC, SHARD, NPAD = meta["WPC"], meta["SHARD"], meta["NPAD"]
    TOTCHUNKS, TOTCOLS = meta["TOTCHUNKS"], meta["TOTCOLS"]
    QWIN, QROWS, QSTART = meta["QWIN"], meta["QROWS"], meta["QSTART"]
    CLS, CLS_BASE = meta["CLS"], meta["CLS_BASE"]
    KIN = IN_C // P

    nc = bacc.Bacc("TRN2", target_bir_lowering=False, debug=False,
                   num_devices=NCORES, num_swdge_queues=4)
    f32, bf16, i16, i32 = (mybir.dt.float32, mybir.dt.bfloat16,
                           mybir.dt.int16, mybir.dt.int32)

    z_shardT = nc.dram_tensor("z_shardT", [IN_C, SHARD], bf16, kind="ExternalInput").ap()
    w1 = nc.dram_tensor("w1", [IN_C, HID], bf16, kind="ExternalInput").ap()
    w2 = nc.dram_tensor("w2", [HID, OUT_C], bf16, kind="ExternalInput").ap()
    idx16 = nc.dram_tensor("idx16", [128, TOTCOLS], i16, kind="ExternalInput").ap()
    dstloc = nc.dram_tensor("dstloc", [128, TOTCHUNKS], bf16, kind="ExternalInput").ap()
    dinv_col = nc.dram_tensor("dinv_col", [P, WPC], f32, kind="ExternalInput").ap()
    sqd_row = nc.dram_tensor("sqd_row", [1, SHARD], bf16, kind="ExternalInput").ap()
    b1r = nc.dram_tensor("b1r", [1, HID], bf16, kind="ExternalInput").ap()
    b2r = nc.dram_tensor("b2r", [1, OUT_C], bf16, kind="ExternalInput").ap()
    out_shard = nc.dram_tensor("out_shard", [SHARD, OUT_C], f32, kind="ExternalOutput").ap()

    # windows after which each AG chunk becomes issuable (last window of
    # each quarter)
    q_end_win = np.cumsum(QWIN) - 1          # e.g. [24, 49, 73, 97]

    with tile.TileContext(nc) as tc:
        with (
            tc.tile_pool(name="dram", bufs=1, space="DRAM") as dram,
            tc.tile_pool(name="const", bufs=1) as cp,
        ):
            ag1_in = dram.tile([SHARD, HID], bf16)
            ag2_in = dram.tile([SHARD, HID], bf16)
            # one Shared tile per AllGather chunk (CoreSim allows only a
            # single writer instruction per Shared DRAM tensor)
            table1 = [dram.tile([CLS[j], HID], bf16, addr_space="Shared",
                                name=f"table1_{j}")
                      for j in range(NCLASS)]
            table2 = [dram.tile([CLS[j], HID], bf16, addr_space="Shared",
                                name=f"table2_{j}")
                      for j in range(NCLASS)]

            w1sb = cp.tile([P, KIN * HID], bf16)
            for ic in range(KIN):
                nc.sync.dma_start(w1sb[:, ic * HID:(ic + 1) * HID],
                                  w1[ic * P:(ic + 1) * P, :])
            w2sb = cp.tile([P, OUT_C], bf16)
            nc.sync.dma_start(w2sb[:], w2[:])
            dinvsb = cp.tile([P, WPC], f32)
            nc.sync.dma_start(dinvsb[:], dinv_col[:])
            sqdsb = cp.tile([1, SHARD], bf16)
            nc.sync.dma_start(sqdsb[:], sqd_row[:])
            b1sb = cp.tile([1, HID], bf16)
            nc.sync.dma_start(b1sb[:], b1r[:])
            b2sb = cp.tile([1, OUT_C], bf16)
            nc.sync.dma_start(b2sb[:], b2r[:])

            iota_i = cp.tile([P, P], i32)
            nc.gpsimd.iota(iota_i[:], pattern=[[1, P]], base=0, channel_multiplier=0)
            iota_bf = cp.tile([P, P], bf16)
            nc.vector.tensor_copy(iota_bf[:], iota_i[:])

            def issue_ag(ag_in, table, j):
                nc.gpsimd.collective_compute(
                    "AllGather", mybir.AluOpType.bypass,
                    replica_groups=[list(range(NCORES))],
                    ins=[ag_in[QSTART[j]:QSTART[j] + QROWS[j], :]],
                    outs=[table[j][:]])

            # ---------------- phase A: h1' = (z @ W1) * dinv  (own shard)
            with (
                tc.tile_pool(name="mmA", bufs=2) as mp,
                tc.tile_pool(name="psA", bufs=2, space="PSUM") as psA,
            ):
                next_q = 0
                for t0 in range(0, SHARD, 512):
                    gsz = min(512, SHARD - t0)
                    zts = []
                    for ic in range(KIN):
                        zt = mp.tile([P, gsz], bf16, tag=f"zt{ic}",
                                     padded_shape=[P, 512], name=f"zt{ic}")
                        nc.sync.dma_start(
                            zt[:], z_shardT[ic * P:(ic + 1) * P, t0:t0 + gsz])
                        zts.append(zt)
                    for sub in range(gsz // P):
                        nt = t0 // P + sub
                        ps = psA.tile([P, HID], f32, name="psa")
                        for ic in range(KIN):
                            nc.tensor.matmul(
                                ps[:], lhsT=zts[ic][:, sub * P:(sub + 1) * P],
                                rhs=w1sb[:, ic * HID:(ic + 1) * HID],
                                start=(ic == 0), stop=(ic == KIN - 1))
                        hsb = mp.tile([P, HID], bf16, tag="hsb", name="hsb")
                        nc.scalar.mul(hsb[:], ps[:], dinvsb[:, nt:nt + 1])
                        nc.sync.dma_start(ag1_in[nt * P:(nt + 1) * P, :], hsb[:])
                        while next_q < NCLASS and nt == q_end_win[next_q]:
                            issue_ag(ag1_in, table1, next_q)
                            next_q += 1

            # ---------------- aggregation layers
            def agg_layer(table, layer):
                next_q = 0
                with (
                    tc.tile_pool(name=f"gat{layer}", bufs=2) as gp,
                    tc.tile_pool(name=f"s{layer}", bufs=3) as sp,
                    tc.tile_pool(name=f"eps{layer}", bufs=3) as ep,
                    tc.tile_pool(name=f"ps{layer}", bufs=2, space="PSUM") as pp,
                    tc.tile_pool(name=f"pso{layer}", bufs=2, space="PSUM") as po,
                ):
                    maxgch = max(gm["nchunks"] for gm in meta["groups"])
                    maxgcol = max(gm["ncols"] for gm in meta["groups"])
                    maxsch = max(len(wm["gchunks"]) for wm in meta["wmeta"])
                    for gm in meta["groups"]:
                        gch, gcol = gm["nchunks"], gm["ncols"]
                        idx_sb = gp.tile([128, gcol], i16, tag="idx",
                                         padded_shape=[128, maxgcol], name="idx_sb")
                        nc.sync.dma_start(idx_sb[:], idx16[:, gm["col_base"]:gm["col_base"] + gcol])
                        dl_sb = gp.tile([P, gch], bf16, tag="dl",
                                        padded_shape=[P, maxgch], name="dl_sb")
                        nc.sync.dma_start(dl_sb[:], dstloc[:, gm["chunk_base"]:gm["chunk_base"] + gch])
                        gbuf = gp.tile([P, gch * P], bf16, tag="gbuf",
                                       padded_shape=[P, maxgch * P], name="gbuf")
                        for call in gm["calls"]:
                            cn = call["nchunks"]
                            c = call["cls"]
                            if cn == 0:
                                continue
                            off = call["chunk_start"] - gm["chunk_base"]
                            loc0 = call["col_start"] - gm["col_base"]
                            # <=1024 idx per call: the single_packet fast path
                            # (64-desc packet x 16 lanes); queue = class so 4
                            # Q7 core pairs generate descriptors in parallel
                            for s0 in range(0, cn, 8):
                                sc = min(8, cn - s0)
                                nc.gpsimd.dma_gather(
                                    out_ap=gbuf[:, (off + s0) * P:(off + s0 + sc) * P]
                                        .rearrange("p (k f) -> p k f", f=P),
                                    in_ap=table[c][:],
                                    idxs_ap=idx_sb[:, loc0 + s0 * 8:loc0 + (s0 + sc) * 8],
                                    num_idxs=sc * P,
                                    num_idxs_reg=sc * P,
                                    elem_size=HID,
                                    single_packet=True,
                                    queue_num=c,
                                )
                        for w in gm["windows"]:
                            wm = meta["wmeta"][w]
                            cw = len(wm["gchunks"])
                            s_sb = sp.tile([P, max(cw, 1) * P], bf16, tag="s",
                                           padded_shape=[P, maxsch * P], name="s_sb")
                            soff = 0
                            for (c, ncw, gbase) in wm["schunks"]:
                                lc0 = gbase - gm["chunk_base"]
                                in0 = (dl_sb[:, lc0:lc0 + ncw]
                                       .rearrange("p (c one) -> p c one", one=1)
                                       .to_broadcast([P, ncw, P]))
                                in1 = (iota_bf[:]
                                       .rearrange("p (one j) -> p one j", one=1)
                                       .to_broadcast([P, ncw, P]))
                                nc.vector.tensor_tensor(
                                    out=s_sb[:, soff * P:(soff + ncw) * P]
                                        .rearrange("p (c j) -> p c j", j=P),
                                    in0=in0, in1=in1,
                                    op=mybir.AluOpType.is_equal)
                                soff += ncw
                            ps = pp.tile([P, P], f32, name="ps")
                            if layer == 1:
                                nc.tensor.matmul(
                                    ps[:], lhsT=sqdsb[:, w * P:(w + 1) * P],
                                    rhs=b1sb[:], start=True, stop=(cw == 0))
                                for j, gc in enumerate(wm["gchunks"]):
                                    lgc = gc - gm["chunk_base"]
                                    nc.tensor.matmul(
                                        ps[:],
                                        lhsT=s_sb[:, j * P:(j + 1) * P],
                                        rhs=gbuf[:, lgc * P:(lgc + 1) * P],
                                        start=False,
                                        stop=(j == cw - 1))
                                l1sb = ep.tile([P, HID], bf16, tag="l1", name="l1sb")
                                nc.scalar.activation(
                                    l1sb[:], ps[:],
                                    mybir.ActivationFunctionType.Relu,
                                    scale=dinvsb[:, w:w + 1])
                                l2row = ep.tile([P, HID], bf16, tag="l2r", name="l2row")
                                # ACT-engine mul: vector.tensor_scalar with a
                                # per-partition scalar column measured ~14us
                                # per window on DVE; ACT does this in ~0.6us
                                nc.scalar.mul(l2row[:], l1sb[:], dinvsb[:, w:w + 1])
                                nc.sync.dma_start(ag2_in[w * P:(w + 1) * P, :], l2row[:])
                                while next_q < NCLASS and w == q_end_win[next_q]:
                                    issue_ag(ag2_in, table2, next_q)
                                    next_q += 1
                            else:
                                # transposed accumulate: ps[f, d]
                                for j, gc in enumerate(wm["gchunks"]):
                                    lgc = gc - gm["chunk_base"]
                                    nc.tensor.matmul(
                                        ps[:],
                                        lhsT=gbuf[:, lgc * P:(lgc + 1) * P],
                                        rhs=s_sb[:, j * P:(j + 1) * P],
                                        start=(j == 0), stop=(j == cw - 1))
                                a2t = ep.tile([P, P], bf16, tag="a2t", name="a2t")
                                if cw == 0:
                                    nc.vector.memset(a2t[:], 0)
                                else:
                                    nc.vector.tensor_copy(a2t[:], ps[:])
                                ops = po.tile([P, OUT_C], f32, name="ops")
                                nc.tensor.matmul(ops[:], lhsT=a2t[:], rhs=w2sb[:],
                                                 start=True, stop=False)
                                nc.tensor.matmul(ops[:], lhsT=sqdsb[:, w * P:(w + 1) * P],
                                                 rhs=b2sb[:], start=False, stop=True)
                                fsb = ep.tile([P, OUT_C], f32, tag="fout", name="fsb")
                                nc.scalar.mul(fsb[:], ops[:], dinvsb[:, w:w + 1])
                                nc.sync.dma_start(out_shard[w * P:(w + 1) * P, :], fsb[:])

            agg_layer(table1, 1)
            agg_layer(table2, 2)

    nc.compile()
    return nc


# ----------------------------------------------------------------- entry


def _prepare_and_build(z, edge_index, W1, b1, W2, b2):
    N, IN_C = z.shape
    HID = W1.shape[1]
    OUT_C = W2.shape[1]
    meta, deg, idx16, dstloc = _plan(edge_index, N)
    WPC, SHARD, NPAD = meta["WPC"], meta["SHARD"], meta["NPAD"]

    dinv = (1.0 / np.sqrt(deg)).astype(np.float32)
    dinv_pad = np.zeros(NPAD, np.float32)
    dinv_pad[:N] = dinv
    sqd_pad = np.zeros(NPAD, np.float32)
    sqd_pad[:N] = np.sqrt(deg).astype(np.float32)

    zpad = np.zeros((NPAD, IN_C), BF16)
    zpad[:N] = z.astype(BF16)

    w1b = np.ascontiguousarray(W1.astype(BF16))
    w2b = np.ascontiguousarray(W2.astype(BF16))
    b1b = np.ascontiguousarray(b1.reshape(1, HID).astype(BF16))
    b2b = np.ascontiguousarray(b2.reshape(1, OUT_C).astype(BF16))

    in_maps = []
    for c in range(NCORES):
        sl = slice(c * SHARD, (c + 1) * SHARD)
        in_maps.append({
            "z_shardT": np.ascontiguousarray(zpad[sl].T),
            "w1": w1b, "w2": w2b,
            "idx16": np.ascontiguousarray(idx16[c]),
            "dstloc": np.ascontiguousarray(dstloc[c]),
            "dinv_col": np.ascontiguousarray(dinv_pad[sl].reshape(WPC, P).T),
            "sqd_row": np.ascontiguousarray(sqd_pad[sl].reshape(1, SHARD).astype(BF16)),
            "b1r": b1b, "b2r": b2b,
        })

    cache_key = (N, IN_C, HID, OUT_C, meta["TOTCHUNKS"], hash(meta["chunks_sig"]))
    if cache_key in _PROGRAM_CACHE:
        nc = _PROGRAM_CACHE[cache_key]
    else:
        nc = _build_program(meta, IN_C, HID, OUT_C)
        _PROGRAM_CACHE[cache_key] = nc
    return nc, in_maps, meta


def _run(inputs, trace=False, trace_kwargs=None):
    from concourse.bass_utils import run_bass_kernel_spmd

    z = np.asarray(inputs["z"])
    edge_index = np.asarray(inputs["edge_index"])
    W1 = np.asarray(inputs["W1"])
    b1 = np.asarray(inputs["b1"])
    W2 = np.asarray(inputs["W2"])
    b2 = np.asarray(inputs["b2"])

    nc, in_maps, meta = _prepare_and_build(z, edge_index, W1, b1, W2, b2)
    res = run_bass_kernel_spmd(
        nc, in_maps, core_ids=list(range(NCORES)),
        trace=trace, **(trace_kwargs or {}))
    N = meta["N"]
    out = np.concatenate([r["out_shard"] for r in res.results], axis=0)[:N]
    return np.ascontiguousarray(out.astype(np.float32)), res


def kernel(**inputs):
    out, _ = _run(inputs, trace=False)
    return out


# revision 13
# speedup vs baseline: 3.4090x; 1.4954x over previous
"""Two-layer GCN (PyG GCNConv x2 with relu between) on 8 Trainium2 NeuronCores.

Math (per layer, A' = D^-1/2 (A + I) D^-1/2):
    h  = relu(A' (z @ W1) + b1)
    out = A' (h @ W2) + b2  ==  (A' h) @ W2 + b2      (aggregation commutes with the
                                                       feature-space linear map)
Both layers therefore aggregate 128-wide features only.

Distribution: nodes (and dst-partitioned edges) sharded across 8 cores;
weights replicated; per-layer AllGather of the (dinv-scaled) feature table in
bf16; per-core gather of source rows via bulk SWDGE dma_gather; segment-sum
realized as one-hot matmuls accumulating in PSUM.

Perf structure (vs the naive version):
  * dma_gather descriptor generation runs on ONE Q7 core pair selected by
    queue_num; with num_swdge_queues=4 and gathers spread round-robin over
    queues 0-3, four core pairs generate descriptors concurrently
    (~2.6 ns/idx instead of ~8.8).
  * The per-layer AllGather is split into 4 class chunks (classes = row
    quarters of each shard, permuted table layout) so gathers for class c
    only wait on AG chunk c, and AG chunks issue as soon as the producing
    quarter of the shard is computed -> collectives overlap compute.

The Bass program is specialized to the actual graph: per-(window, class)
chunk counts are compile-time constants derived from edge_index.
"""

import numpy as np
import ml_dtypes

P = 128
NCORES = 8
NCLASS = 4          # src-range classes (= AllGather chunks); rel idx fits int16
G = 4               # dst windows per gather group

BF16 = ml_dtypes.bfloat16

_PROGRAM_CACHE = {}


# ----------------------------------------------------------------- host prep


def _plan(edge_index, N):
    """Sort/partition edges; all compile-time metadata + per-core slot arrays."""
    WPC = -(-N // (NCORES * P))            # windows per core
    SHARD = WPC * P
    NPAD = NCORES * SHARD

    # quarters of each shard in WINDOWS (so AG chunk boundaries align with
    # aggregation window groups); table layout is class-major:
    #   node (core c, local row r) with r in quarter j sits at table row
    #   CLS_BASE[j] + c * QROWS[j] + (r - QSTART[j])
    wq = WPC // NCLASS
    QWIN = [wq + (1 if j < WPC % NCLASS else 0) for j in range(NCLASS)]
    QROWS = [w * P for w in QWIN]
    QSTART = np.concatenate([[0], np.cumsum(QROWS)]).astype(np.int64)
    CLS = [NCORES * q for q in QROWS]
    CLS_BASE = np.concatenate([[0], np.cumsum(CLS)]).astype(np.int64)
    assert max(CLS) <= 32768

    src = np.asarray(edge_index[0], dtype=np.int64)
    dst = np.asarray(edge_index[1], dtype=np.int64)
    deg = np.bincount(dst, minlength=N).astype(np.float64) + 1.0

    loops = np.arange(N, dtype=np.int64)
    s2 = np.concatenate([src, loops])
    d2 = np.concatenate([dst, loops])

    # class + within-class (permuted-table) index of each source node
    s_core = s2 // SHARD
    s_r = s2 % SHARD
    s_cls = np.searchsorted(QSTART, s_r, side="right") - 1      # quarter of r
    s_rel = s_core * np.array(QROWS)[s_cls] + (s_r - QSTART[s_cls])

    win = d2 >> 7
    key = win * NCLASS + s_cls
    order = np.argsort(key, kind="stable")
    rel_s = s_rel[order]
    d2s = d2[order]

    NW = NPAD // P
    cellcnt = np.bincount(key, minlength=NW * NCLASS)
    cellstart = np.concatenate([[0], np.cumsum(cellcnt)]).astype(np.int64)
    counts_core = cellcnt.reshape(NCORES, WPC, NCLASS)
    chunks_wc = -(-counts_core.max(axis=0) // P)      # [WPC, NCLASS]

    groups = [list(range(g, min(g + G, WPC))) for g in range(0, WPC, G)]

    # global chunk layout: for each group, for each class, for each window in
    # group, that window's class chunks (sub-calls of <=8 chunks per
    # (group, class); queue_num = class).
    group_meta = []           # per group: dict with chunk/col offsets
    wmeta = [dict(schunks=[], gchunks=[]) for _ in range(WPC)]
    chunkpos = 0
    colpos = 0
    cell_slot = {}            # (w, c) -> global slot start
    for grp in groups:
        g_chunk_base = chunkpos
        g_col_base = colpos
        calls = []
        for c in range(NCLASS):
            call_chunk_start = chunkpos
            call_col_start = colpos
            for w in grp:
                ncw = int(chunks_wc[w, c])
                cell_slot[(w, c)] = chunkpos * P
                chunkpos += ncw
            cn = chunkpos - call_chunk_start
            colpos += cn * P // 16
            calls.append(dict(chunk_start=call_chunk_start, nchunks=cn,
                              col_start=call_col_start, ncols=colpos - call_col_start,
                              cls=c))
        group_meta.append(dict(chunk_base=g_chunk_base, nchunks=chunkpos - g_chunk_base,
                               col_base=g_col_base, ncols=colpos - g_col_base,
                               calls=calls, windows=list(grp)))
    TOTCHUNKS = chunkpos
    TOTCOLS = colpos

    # per-window ordered chunk lists: s-order (class-major) + matching global
    # chunk ids, and per-(w,c) count for S generation
    for w in range(WPC):
        for c in range(NCLASS):
            ncw = int(chunks_wc[w, c])
            if ncw == 0:
                continue
            base = cell_slot[(w, c)] // P
            wmeta[w]["gchunks"].extend(range(base, base + ncw))
            wmeta[w]["schunks"].append((c, ncw, base))

    # per-core slot arrays
    idx_slots = np.zeros((NCORES, TOTCHUNKS * P), np.int16)
    dst_slots = np.full((NCORES, TOTCHUNKS * P), 300.0, np.float32)
    for w in range(WPC):
        for c in range(NCLASS):
            ncw = int(chunks_wc[w, c])
            if ncw == 0:
                continue
            s0 = cell_slot[(w, c)]
            for core in range(NCORES):
                cidx = (core * WPC + w) * NCLASS + c
                cnt = int(cellcnt[cidx])
                st = int(cellstart[cidx])
                idx_slots[core, s0:s0 + cnt] = rel_s[st:st + cnt].astype(np.int16)
                dst_slots[core, s0:s0 + cnt] = (d2s[st:st + cnt] & (P - 1)).astype(np.float32)

    # wrapped int16 index tensors (per call: idx i at [i%16, i//16], tiled x8)
    idx16 = np.zeros((NCORES, 128, TOTCOLS), np.int16)
    for gm in group_meta:
        for call in gm["calls"]:
            cn = call["nchunks"]
            if cn == 0:
                continue
            s0 = call["chunk_start"] * P
            c0 = call["col_start"]
            seg = idx_slots[:, s0:s0 + cn * P]                  # [NCORES, n]
            wrapped = seg.reshape(NCORES, cn * P // 16, 16).transpose(0, 2, 1)
            idx16[:, :, c0:c0 + cn * P // 16] = np.tile(wrapped, (1, 8, 1))

    dstloc = dst_slots.reshape(NCORES, TOTCHUNKS, P).transpose(0, 2, 1)  # [NCORES,128,TOTCHUNKS]

    meta = dict(N=N, WPC=WPC, SHARD=SHARD, NPAD=NPAD,
                QWIN=QWIN, QROWS=QROWS, QSTART=[int(x) for x in QSTART],
                CLS=CLS, CLS_BASE=[int(x) for x in CLS_BASE],
                TOTCHUNKS=TOTCHUNKS, TOTCOLS=TOTCOLS,
                groups=group_meta, wmeta=wmeta,
                chunks_sig=chunks_wc.tobytes())
    return meta, deg, idx16, dstloc.astype(BF16)


# ------------------------------------------------------------- bass program


def _build_program(meta, IN_C, HID, OUT_C):
    import concourse.bacc as bacc
    import concourse.mybir as mybir
    import concourse.tile as tile

    WPC, SHARD, NPAD = meta["WPC"], meta["SHARD"], meta["NPAD"]
    TOTCHUNKS, TOTCOLS = meta["TOTCHUNKS"], meta["TOTCOLS"]
    QWIN, QROWS, QSTART = meta["QWIN"], meta["QROWS"], meta["QSTART"]
    CLS, CLS_BASE = meta["CLS"], meta["CLS_BASE"]
    KIN = IN_C // P

    nc = bacc.Bacc("TRN2", target_bir_lowering=False, debug=False,
                   num_devices=NCORES, num_swdge_queues=4)
    f32, bf16, i16, i32 = (mybir.dt.float32, mybir.dt.bfloat16,
                           mybir.dt.int16, mybir.dt.int32)

    z_shardT = nc.dram_tensor("z_shardT", [IN_C, SHARD], bf16, kind="ExternalInput").ap()
    w1 = nc.dram_tensor("w1", [IN_C, HID], bf16, kind="ExternalInput").ap()
    w2 = nc.dram_tensor("w2", [HID, OUT_C], bf16, kind="ExternalInput").ap()
    idx16 = nc.dram_tensor("idx16", [128, TOTCOLS], i16, kind="ExternalInput").ap()
    dstloc = nc.dram_tensor("dstloc", [128, TOTCHUNKS], bf16, kind="ExternalInput").ap()
    dinv_col = nc.dram_tensor("dinv_col", [P, WPC], f32, kind="ExternalInput").ap()
    sqd_row = nc.dram_tensor("sqd_row", [1, SHARD], bf16, kind="ExternalInput").ap()
    b1r = nc.dram_tensor("b1r", [1, HID], bf16, kind="ExternalInput").ap()
    b2r = nc.dram_tensor("b2r", [1, OUT_C], bf16, kind="ExternalInput").ap()
    out_shard = nc.dram_tensor("out_shard", [SHARD, OUT_C], f32, kind="ExternalOutput").ap()

    # windows after which each AG chunk becomes issuable (last window of
    # each quarter)
    q_end_win = np.cumsum(QWIN) - 1          # e.g. [24, 49, 73, 97]

    with tile.TileContext(nc) as tc:
        with (
            tc.tile_pool(name="dram", bufs=1, space="DRAM") as dram,
            tc.tile_pool(name="const", bufs=1) as cp,
        ):
            ag1_in = dram.tile([SHARD, HID], bf16)
            ag2_in = dram.tile([SHARD, HID], bf16)
            # one Shared tile per AllGather chunk (CoreSim allows only a
            # single writer instruction per Shared DRAM tensor)
            table1 = [dram.tile([CLS[j], HID], bf16, addr_space="Shared",
                                name=f"table1_{j}")
                      for j in range(NCLASS)]
            table2 = [dram.tile([CLS[j], HID], bf16, addr_space="Shared",
                                name=f"table2_{j}")
                      for j in range(NCLASS)]

            w1sb = cp.tile([P, KIN * HID], bf16)
            for ic in range(KIN):
                nc.sync.dma_start(w1sb[:, ic * HID:(ic + 1) * HID],
                                  w1[ic * P:(ic + 1) * P, :])
            w2sb = cp.tile([P, OUT_C], bf16)
            nc.sync.dma_start(w2sb[:], w2[:])
            dinvsb = cp.tile([P, WPC], f32)
            nc.sync.dma_start(dinvsb[:], dinv_col[:])
            sqdsb = cp.tile([1, SHARD], bf16)
            nc.sync.dma_start(sqdsb[:], sqd_row[:])
            b1sb = cp.tile([1, HID], bf16)
            nc.sync.dma_start(b1sb[:], b1r[:])
            b2sb = cp.tile([1, OUT_C], bf16)
            nc.sync.dma_start(b2sb[:], b2r[:])

            iota_i = cp.tile([P, P], i32)
            nc.gpsimd.iota(iota_i[:], pattern=[[1, P]], base=0, channel_multiplier=0)
            iota_bf = cp.tile([P, P], bf16)
            nc.vector.tensor_copy(iota_bf[:], iota_i[:])

            def issue_ag(ag_in, table, j):
                nc.gpsimd.collective_compute(
                    "AllGather", mybir.AluOpType.bypass,
                    replica_groups=[list(range(NCORES))],
                    ins=[ag_in[QSTART[j]:QSTART[j] + QROWS[j], :]],
                    outs=[table[j][:]])

            # ---------------- phase A: h1' = (z @ W1) * dinv  (own shard)
            with (
                tc.tile_pool(name="mmA", bufs=2) as mp,
                tc.tile_pool(name="psA", bufs=2, space="PSUM") as psA,
            ):
                next_q = 0
                for t0 in range(0, SHARD, 512):
                    gsz = min(512, SHARD - t0)
                    zts = []
                    for ic in range(KIN):
                        zt = mp.tile([P, gsz], bf16, tag=f"zt{ic}",
                                     padded_shape=[P, 512], name=f"zt{ic}")
                        nc.sync.dma_start(
                            zt[:], z_shardT[ic * P:(ic + 1) * P, t0:t0 + gsz])
                        zts.append(zt)
                    for sub in range(gsz // P):
                        nt = t0 // P + sub
                        ps = psA.tile([P, HID], f32, name="psa")
                        for ic in range(KIN):
                            nc.tensor.matmul(
                                ps[:], lhsT=zts[ic][:, sub * P:(sub + 1) * P],
                                rhs=w1sb[:, ic * HID:(ic + 1) * HID],
                                start=(ic == 0), stop=(ic == KIN - 1))
                        hsb = mp.tile([P, HID], bf16, tag="hsb", name="hsb")
                        nc.scalar.mul(hsb[:], ps[:], dinvsb[:, nt:nt + 1])
                        nc.sync.dma_start(ag1_in[nt * P:(nt + 1) * P, :], hsb[:])
                        while next_q < NCLASS and nt == q_end_win[next_q]:
                            issue_ag(ag1_in, table1, next_q)
                            next_q += 1

            # ---------------- aggregation layers
            def agg_layer(table, layer):
                next_q = 0
                with (
                    tc.tile_pool(name=f"gat{layer}", bufs=2) as gp,
                    tc.tile_pool(name=f"s{layer}", bufs=3) as sp,
                    tc.tile_pool(name=f"eps{layer}", bufs=3) as ep,
                    tc.tile_pool(name=f"ps{layer}", bufs=2, space="PSUM") as pp,
                    tc.tile_pool(name=f"pso{layer}", bufs=2, space="PSUM") as po,
                ):
                    maxgch = max(gm["nchunks"] for gm in meta["groups"])
                    maxgcol = max(gm["ncols"] for gm in meta["groups"])
                    maxsch = max(len(wm["gchunks"]) for wm in meta["wmeta"])
                    for gm in meta["groups"]:
                        gch, gcol = gm["nchunks"], gm["ncols"]
                        idx_sb = gp.tile([128, gcol], i16, tag="idx",
                                         padded_shape=[128, maxgcol], name="idx_sb")
                        nc.sync.dma_start(idx_sb[:], idx16[:, gm["col_base"]:gm["col_base"] + gcol])
                        dl_sb = gp.tile([P, gch], bf16, tag="dl",
                                        padded_shape=[P, maxgch], name="dl_sb")
                        nc.sync.dma_start(dl_sb[:], dstloc[:, gm["chunk_base"]:gm["chunk_base"] + gch])
                        gbuf = gp.tile([P, gch * P], bf16, tag="gbuf",
                                       padded_shape=[P, maxgch * P], name="gbuf")
                        # <=1024 idx per call: the single_packet fast path
                        # (64-desc packet x 16 lanes); queue = class so 4 Q7
                        # core pairs generate descriptors in parallel. Emit
                        # sub-calls round-robin across classes: consecutive
                        # same-queue instructions stall the in-order Pool
                        # dispatch (exec-queue depth 4) and kill the overlap.
                        subcalls = []
                        for call in gm["calls"]:
                            cn = call["nchunks"]
                            c = call["cls"]
                            if cn == 0:
                                continue
                            off = call["chunk_start"] - gm["chunk_base"]
                            loc0 = call["col_start"] - gm["col_base"]
                            for s0 in range(0, cn, 8):
                                sc = min(8, cn - s0)
                                subcalls.append((c, off + s0, loc0 + s0 * 8, sc))
                        order = []
                        byq = {}
                        for scall in subcalls:
                            byq.setdefault(scall[0], []).append(scall)
                        for i in range(max(len(v) for v in byq.values())):
                            for c in sorted(byq):
                                if i < len(byq[c]):
                                    order.append(byq[c][i])
                        for (c, choff, col0, sc) in order:
                            nc.gpsimd.dma_gather(
                                out_ap=gbuf[:, choff * P:(choff + sc) * P]
                                    .rearrange("p (k f) -> p k f", f=P),
                                in_ap=table[c][:],
                                idxs_ap=idx_sb[:, col0:col0 + sc * 8],
                                num_idxs=sc * P,
                                num_idxs_reg=sc * P,
                                elem_size=HID,
                                single_packet=True,
                                queue_num=c,
                            )
                        for w in gm["windows"]:
                            wm = meta["wmeta"][w]
                            cw = len(wm["gchunks"])
                            s_sb = sp.tile([P, max(cw, 1) * P], bf16, tag="s",
                                           padded_shape=[P, maxsch * P], name="s_sb")
                            soff = 0
                            for (c, ncw, gbase) in wm["schunks"]:
                                lc0 = gbase - gm["chunk_base"]
                                in0 = (dl_sb[:, lc0:lc0 + ncw]
                                       .rearrange("p (c one) -> p c one", one=1)
                                       .to_broadcast([P, ncw, P]))
                                in1 = (iota_bf[:]
                                       .rearrange("p (one j) -> p one j", one=1)
                                       .to_broadcast([P, ncw, P]))
                                nc.vector.tensor_tensor(
                                    out=s_sb[:, soff * P:(soff + ncw) * P]
                                        .rearrange("p (c j) -> p c j", j=P),
                                    in0=in0, in1=in1,
                                    op=mybir.AluOpType.is_equal)
                                soff += ncw
                            ps = pp.tile([P, P], f32, name="ps")
                            if layer == 1:
                                nc.tensor.matmul(
                                    ps[:], lhsT=sqdsb[:, w * P:(w + 1) * P],
                                    rhs=b1sb[:], start=True, stop=(cw == 0))
                                for j, gc in enumerate(wm["gchunks"]):
                                    lgc = gc - gm["chunk_base"]
                                    nc.tensor.matmul(
                                        ps[:],
                                        lhsT=s_sb[:, j * P:(j + 1) * P],
                                        rhs=gbuf[:, lgc * P:(lgc + 1) * P],
                                        start=False,
                                        stop=(j == cw - 1))
                                l1sb = ep.tile([P, HID], bf16, tag="l1", name="l1sb")
                                nc.scalar.activation(
                                    l1sb[:], ps[:],
                                    mybir.ActivationFunctionType.Relu,
                                    scale=dinvsb[:, w:w + 1])
                                l2row = ep.tile([P, HID], bf16, tag="l2r", name="l2row")
                                # ACT-engine mul: vector.tensor_scalar with a
                                # per-partition scalar column measured ~14us
                                # per window on DVE; ACT does this in ~0.6us
                                nc.scalar.mul(l2row[:], l1sb[:], dinvsb[:, w:w + 1])
                                nc.sync.dma_start(ag2_in[w * P:(w + 1) * P, :], l2row[:])
                                while next_q < NCLASS and w == q_end_win[next_q]:
                                    issue_ag(ag2_in, table2, next_q)
                                    next_q += 1
                            else:
                                # transposed accumulate: ps[f, d]
                                for j, gc in enumerate(wm["gchunks"]):
                                    lgc = gc - gm["chunk_base"]
                                    nc.tensor.matmul(
                                        ps[:],
                                        lhsT=gbuf[:, lgc * P:(lgc + 1) * P],
                                        rhs=s_sb[:, j * P:(j + 1) * P],
                                        start=(j == 0), stop=(j == cw - 1))
                                a2t = ep.tile([P, P], bf16, tag="a2t", name="a2t")
                                if cw == 0:
                                    nc.vector.memset(a2t[:], 0)
                                else:
                                    nc.vector.tensor_copy(a2t[:], ps[:])
                                ops = po.tile([P, OUT_C], f32, name="ops")
                                nc.tensor.matmul(ops[:], lhsT=a2t[:], rhs=w2sb[:],
                                                 start=True, stop=False)
                                nc.tensor.matmul(ops[:], lhsT=sqdsb[:, w * P:(w + 1) * P],
                                                 rhs=b2sb[:], start=False, stop=True)
                                fsb = ep.tile([P, OUT_C], f32, tag="fout", name="fsb")
                                nc.scalar.mul(fsb[:], ops[:], dinvsb[:, w:w + 1])
                                nc.sync.dma_start(out_shard[w * P:(w + 1) * P, :], fsb[:])

            agg_layer(table1, 1)
            agg_layer(table2, 2)

    nc.compile()
    return nc


# ----------------------------------------------------------------- entry


def _prepare_and_build(z, edge_index, W1, b1, W2, b2):
    N, IN_C = z.shape
    HID = W1.shape[1]
    OUT_C = W2.shape[1]
    meta, deg, idx16, dstloc = _plan(edge_index, N)
    WPC, SHARD, NPAD = meta["WPC"], meta["SHARD"], meta["NPAD"]

    dinv = (1.0 / np.sqrt(deg)).astype(np.float32)
    dinv_pad = np.zeros(NPAD, np.float32)
    dinv_pad[:N] = dinv
    sqd_pad = np.zeros(NPAD, np.float32)
    sqd_pad[:N] = np.sqrt(deg).astype(np.float32)

    zpad = np.zeros((NPAD, IN_C), BF16)
    zpad[:N] = z.astype(BF16)

    w1b = np.ascontiguousarray(W1.astype(BF16))
    w2b = np.ascontiguousarray(W2.astype(BF16))
    b1b = np.ascontiguousarray(b1.reshape(1, HID).astype(BF16))
    b2b = np.ascontiguousarray(b2.reshape(1, OUT_C).astype(BF16))

    in_maps = []
    for c in range(NCORES):
        sl = slice(c * SHARD, (c + 1) * SHARD)
        in_maps.append({
            "z_shardT": np.ascontiguousarray(zpad[sl].T),
            "w1": w1b, "w2": w2b,
            "idx16": np.ascontiguousarray(idx16[c]),
            "dstloc": np.ascontiguousarray(dstloc[c]),
            "dinv_col": np.ascontiguousarray(dinv_pad[sl].reshape(WPC, P).T),
            "sqd_row": np.ascontiguousarray(sqd_pad[sl].reshape(1, SHARD).astype(BF16)),
            "b1r": b1b, "b2r": b2b,
        })

    cache_key = (N, IN_C, HID, OUT_C, meta["TOTCHUNKS"], hash(meta["chunks_sig"]))
    if cache_key in _PROGRAM_CACHE:
        nc = _PROGRAM_CACHE[cache_key]
    else:
        nc = _build_program(meta, IN_C, HID, OUT_C)
        _PROGRAM_CACHE[cache_key] = nc
    return nc, in_maps, meta


def _run(inputs, trace=False, trace_kwargs=None):
    from concourse.bass_utils import run_bass_kernel_spmd

    z = np.asarray(inputs["z"])
    edge_index = np.asarray(inputs["edge_index"])
    W1 = np.asarray(inputs["W1"])
    b1 = np.asarray(inputs["b1"])
    W2 = np.asarray(inputs["W2"])
    b2 = np.asarray(inputs["b2"])

    nc, in_maps, meta = _prepare_and_build(z, edge_index, W1, b1, W2, b2)
    res = run_bass_kernel_spmd(
        nc, in_maps, core_ids=list(range(NCORES)),
        trace=trace, **(trace_kwargs or {}))
    N = meta["N"]
    out = np.concatenate([r["out_shard"] for r in res.results], axis=0)[:N]
    return np.ascontiguousarray(out.astype(np.float32)), res


def kernel(**inputs):
    out, _ = _run(inputs, trace=False)
    return out
